# revision 16
# baseline (speedup 1.0000x reference)
"""Trainium2 Bass kernel for LinearSelfAttention3D (16x256x64x64, 8 heads, mem_kv).

Data-parallel over batch: 2 batches per core, 8 cores, identical SPMD program.
Per batch (x viewed [256, 4096] channel-major):
  Pass A (32 s-tiles of 128, ctx matmuls software-pipelined one tile behind):
    kT,vT = x^T @ w_{k,v}^T on PE (lhsT = x c-tiles -> [s,d] layout, zero transposes)
    expk = exp(kT) (ACT, fp32r); vT staged with ones-cols (DVE)
    context accumulated in PSUM: 4 head-pair tiles [128, 258]
      (2 heads per tile; col 256 accumulates Z = sum_s expk)
    mem_kv folded on host; added via one identity-matmul [ctxU_mem | Zmem]
    evac: ctx_diag[p] = blockdiag(ctx/Z) fp32r SBUF (batched approx reciprocal)
  Pass B (8 s-chunks of 512, second half pipelined one chunk behind):
    q = w_q @ x (PE, [d,s] layout); expU = exp(q) (ACT)
    Zq[h,s] via indicator matmul; 1/Zq via reciprocal_approx_fast (DVE)
    Bcast = SCALE/Zq broadcast over rows via selector matmul (PE)
    expq = expU * Bcast (DVE); attn_out = ctx_diag^T @ expq (PE)
    y = w_out'^T.T @ attn_out + b' (PE; BatchNorm folded into w_out'/b' on host)
All matmuls fp16 (1 cycle/row, FWL weight loads, ~1.5e-3 rel err).
"""
import os
import sys

sys.path.insert(0, "/opt/trn_rl_repo")
import numpy as np

import concourse.bass as bass  # noqa: E402
import concourse.bacc as bacc  # noqa: E402
import concourse.mybir as mybir  # noqa: E402
import concourse.tile as tile  # noqa: E402
from concourse import bass_utils  # noqa: E402

B, C, H, W = 16, 256, 64, 64
S = H * W  # 4096
MD, NH, HD, NM = 512, 8, 64, 4
SCALE = HD ** -0.5
EPS = 1e-5
N_CORES = 8
BPC = B // N_CORES
NCT = C // 128
NST = S // 128
NSC = S // 512
NDT = MD // 128
F32 = mybir.dt.float32
F16 = mybir.dt.float16
AF = mybir.ActivationFunctionType

_MODULE_CACHE = {}


def _build_module(has_bk, has_bv):
    nc = bacc.Bacc(
        "TRN2",
        target_bir_lowering=False,
        debug=False,
        enable_asserts=False,
        num_devices=N_CORES,
    )
    x_d = nc.dram_tensor("x", (BPC, NCT, 128, S), F16, kind="ExternalInput").ap()
    wqkvT_d = nc.dram_tensor("wqkvT", (NCT, 128, 3 * MD), F16, kind="ExternalInput").ap()
    woutT_d = nc.dram_tensor("woutT", (NDT, 128, C), F16, kind="ExternalInput").ap()
    bq_d = nc.dram_tensor("bq", (128, NDT), F32, kind="ExternalInput").ap()
    bout_d = nc.dram_tensor("bout", (128, 2), F32, kind="ExternalInput").ap()
    bones_d = nc.dram_tensor("bones", (128, 128), F16, kind="ExternalInput").ap()
    cmem_d = nc.dram_tensor("cmem", (128, NDT, 258), F16, kind="ExternalInput").ap()
    ident_d = nc.dram_tensor("ident", (128, 128), F16, kind="ExternalInput").ap()
    y_d = nc.dram_tensor("y", (BPC, 2, 128, S), F16, kind="ExternalOutput").ap()
    if has_bk or has_bv:
        onesrow_d = nc.dram_tensor("onesrow", (2, 128), F16, kind="ExternalInput").ap()
        bkv_d = nc.dram_tensor("bkv", (2, 2 * MD), F16, kind="ExternalInput").ap()

    with tile.TileContext(nc) as tc, nc.allow_low_precision(reason="fp16 matmul operands"):
        import contextlib

        cstack = contextlib.ExitStack()
        const = cstack.enter_context(tc.tile_pool(name="const", bufs=1))
        xrp = cstack.enter_context(tc.tile_pool(name="xrp", bufs=2))
        work = cstack.enter_context(tc.tile_pool(name="work", bufs=3))
        ctxdp = cstack.enter_context(tc.tile_pool(name="ctxdp", bufs=8))
        pool9 = cstack.enter_context(tc.tile_pool(name="pool9", bufs=9))
        pool4 = cstack.enter_context(tc.tile_pool(name="pool4", bufs=4))
        pool2 = cstack.enter_context(tc.tile_pool(name="pool2", bufs=2))

        def load_r(shape, src_ap, tag, n_splits=None):
            t = const.tile(list(shape), F16, tag=tag, name=tag)
            if n_splits is None:
                nc.sync.dma_start(t[:], src_ap)
            else:
                for i in range(n_splits):
                    nc.sync.dma_start(t[:, i], src_ap[i])
            return t

        wq_r = load_r((128, NCT, 3 * MD), wqkvT_d, "wq", n_splits=NCT)
        wo_r = load_r((128, NDT, C), woutT_d, "wo", n_splits=NDT)
        bones_r = load_r((128, 128), bones_d, "bones")
        cmem_r = load_r((128, NDT, 258), cmem_d, "cmem")
        ident_r = load_r((128, 128), ident_d, "ident")
        if has_bk or has_bv:
            onesrow_r = load_r((2, 128), onesrow_d, "onesrow")
            bkv_r = load_r((2, 2 * MD), bkv_d, "bkv")

        bq_t = const.tile([128, NDT], F32, tag="bq")
        nc.sync.dma_start(bq_t[:], bq_d)
        bout_t = const.tile([128, 2], F32, tag="bout")
        nc.sync.dma_start(bout_t[:], bout_d)

        zero_r = const.tile([128, 128], F16, tag="zeror")
        nc.vector.memset(zero_r[:], 0.0)
        ones_r = const.tile([128, 4], F16, tag="onesr")
        nc.vector.memset(ones_r[:], 1.0)

        wzero = const.tile([128, 512], F16, tag="wzero")
        nc.vector.memset(wzero[:], 0.0)
        # HAM warmup: dense matmuls with no DMA dependency (overlap input DMAs)
        with tc.tile_pool(name="warm", bufs=1, space="PSUM") as pw:
            wps = pw.tile([128, 512], F32, tag="warm")
            for i in range(24):
                nc.tensor.matmul(wps[:], zero_r[:], wzero[:],
                                 start=True, stop=True)

        for b in range(BPC):
            x_r = xrp.tile([128, NCT, S], F16, tag="xr")
            for ct in range(NCT):
                nc.sync.dma_start(x_r[:, ct, :], x_d[b, ct])

            # ---- pass A ----
            ctxd = []
            with tc.tile_pool(name=f"pp{b}", bufs=1, space="PSUM") as pp:
                ctxps = [
                    pp.tile([128, 258], F32, tag=f"s{p}", name=f"ctx{p}_{b}")
                    for p in range(NDT)
                ]

                def ctx_mms(ek, vt, st):
                    for p in range(NDT):
                        nc.tensor.matmul(
                            ctxps[p][:],
                            ek[:, p * 128:(p + 1) * 128],
                            vt[:, p // 2, :],
                            start=(st == 0),
                            stop=False,
                        )

                prev = None
                for st in range(NST):
                    kps = pp.tile([128, 512], F32, tag=f"s{4 + (2 * st) % 4}",
                                  name=f"k_{b}_{st}")
                    vps = pp.tile([128, 512], F32, tag=f"s{4 + (2 * st + 1) % 4}",
                                  name=f"v_{b}_{st}")
                    # interleaved so consecutive matmuls share the stationary x tile
                    for ct in range(NCT):
                        last = ct == NCT - 1
                        nc.tensor.matmul(
                            kps[:],
                            x_r[:, ct, st * 128:(st + 1) * 128],
                            wq_r[:, ct, MD:2 * MD],
                            start=(ct == 0),
                            stop=(last and not has_bk),
                        )
                        nc.tensor.matmul(
                            vps[:],
                            x_r[:, ct, st * 128:(st + 1) * 128],
                            wq_r[:, ct, 2 * MD:3 * MD],
                            start=(ct == 0),
                            stop=(last and not has_bv),
                        )
                    if has_bk:
                        nc.tensor.matmul(kps[:], onesrow_r[:], bkv_r[:, 0:MD],
                                         start=False, stop=True)
                    if has_bv:
                        nc.tensor.matmul(vps[:], onesrow_r[:], bkv_r[:, MD:2 * MD],
                                         start=False, stop=True)
                    if prev is not None:
                        ctx_mms(*prev)
                    ek = work.tile([128, 512], F16, tag="ek")
                    nc.scalar.activation(ek[:], kps[:], AF.Exp)
                    vt = work.tile([128, 2, 258], F16, tag="vt")
                    nc.vector.tensor_copy(vt[:, 0, 0:256], vps[:, 0:256])
                    nc.vector.tensor_copy(vt[:, 1, 0:256], vps[:, 256:512])
                    nc.vector.tensor_copy(
                        vt[:, :, 256:258], ones_r[:].rearrange("p (g c) -> p g c", g=2)
                    )
                    prev = (ek, vt, st)
                ctx_mms(*prev)
                for p in range(NDT):
                    nc.tensor.matmul(ctxps[p][:], ident_r[:], cmem_r[:, p, :],
                                     start=False, stop=True)
                # ctx evac: batched 1/Z then per-head blockdiag scale
                zcat = pool4.tile([128, 4], F32, tag="zcat")
                for p in range(NDT):
                    nc.vector.tensor_copy(zcat[:, p:p + 1], ctxps[p][:, 256:257])
                rz = pool4.tile([128, 4], F32, tag="rz")
                nc.vector.reciprocal_approx_fast(rz[:], zcat[:])
                for p in range(NDT):
                    cd = ctxdp.tile([128, 128], F16, tag="ctxd", name=f"cd{p}_{b}")
                    base = (p % 2) * 128
                    nc.vector.tensor_copy(cd[0:64, 64:128], zero_r[0:64, 0:64])
                    nc.vector.tensor_copy(cd[64:128, 0:64], zero_r[0:64, 0:64])
                    nc.vector.tensor_scalar_mul(
                        cd[0:64, 0:64], ctxps[p][0:64, base:base + 64], rz[0:64, p:p + 1]
                    )
                    nc.vector.tensor_scalar_mul(
                        cd[64:128, 64:128],
                        ctxps[p][64:128, base + 64:base + 128],
                        rz[64:128, p:p + 1],
                    )
                    ctxd.append(cd)
                # ---- pass B ---- (3-stage pipeline: [q+stats] -> [attn out] -> [y])

                def stage_ao(sc, eus, rbs):
                    outs = []
                    for d in range(NDT):
                        ao = pp.tile([128, 512], F32, tag=f"s{d % 2}",
                                     name=f"ao{b}_{sc}_{d}")
                        nc.tensor.matmul(ao[:], ctxd[d][:], eus[d][:],
                                         start=True, stop=True)
                        osb = pool9.tile([128, 512], F16, tag="aosb")
                        nc.vector.tensor_mul(osb[:], rbs[d][:], ao[:])
                        outs.append(osb)
                    return outs

                def stage_y(sc, outs):
                    s0 = sc * 512
                    for ot in range(2):
                        yps = pp.tile([128, 512], F32, tag=f"s{7 - 4 * ot}",
                                      name=f"y{b}_{sc}_{ot}")
                        for j in range(NDT):
                            nc.tensor.matmul(
                                yps[:],
                                wo_r[:, j, ot * 128:(ot + 1) * 128],
                                outs[j][:],
                                start=(j == 0),
                                stop=(j == NDT - 1),
                            )
                        ysb = pool4.tile([128, 512], F16, tag="ysb")
                        nc.scalar.activation(ysb[:], yps[:], AF.Identity,
                                             bias=bout_t[:, ot:ot + 1])
                        nc.sync.dma_start(y_d[b, ot, :, s0:s0 + 512], ysb[:])

                st1 = None  # (sc, eus, rbs) awaiting attn-out
                st2 = None  # (sc, outs) awaiting y projection
                for sc in range(NSC):
                    s0 = sc * 512
                    eus = []
                    rbs = []
                    for d in range(NDT):
                        qps = pp.tile([128, 512], F32, tag=f"s{5 + d % 2}",
                                      name=f"q{b}_{sc}_{d}")
                        for ct in range(NCT):
                            nc.tensor.matmul(
                                qps[:],
                                wq_r[:, ct, d * 128:(d + 1) * 128],
                                x_r[:, ct, s0:s0 + 512],
                                start=(ct == 0),
                                stop=(ct == NCT - 1),
                            )
                        eu = pool9.tile([128, 512], F16, tag="eu")
                        nc.scalar.activation(eu[:], qps[:], AF.Exp,
                                             bias=bq_t[:, d:d + 1])
                        # Zq in row-broadcast layout via block-diag ones matmul
                        zqb = pp.tile([128, 512], F32, tag=f"s{2 + 2 * (d % 2)}",
                                      name=f"zqb{b}_{sc}_{d}")
                        nc.tensor.matmul(zqb[:], bones_r[:], eu[:],
                                         start=True, stop=True)
                        rb = pool9.tile([128, 512], F32, tag="rb")
                        nc.vector.reciprocal_approx_fast(rb[:], zqb[:])
                        eus.append(eu)
                        rbs.append(rb)
                    if st2 is not None:
                        stage_y(*st2)
                    if st1 is not None:
                        st2 = (st1[0], stage_ao(*st1))
                    st1 = (sc, eus, rbs)
                if st2 is not None:
                    stage_y(*st2)
                st2 = (st1[0], stage_ao(*st1))
                stage_y(*st2)
        cstack.close()

    nc.compile()
    return nc


def _prep_consts(w_qkv, b_qkv, mem_kv, w_out, b_out, bn_gamma, bn_beta, bn_mean, bn_var):
    w_qkv = np.asarray(w_qkv, np.float32)
    b_qkv = np.asarray(b_qkv, np.float32)
    mem_kv = np.asarray(mem_kv, np.float32)
    w_out = np.asarray(w_out, np.float32)
    b_out = np.asarray(b_out, np.float32)
    g = np.asarray(bn_gamma, np.float64)
    be = np.asarray(bn_beta, np.float64)
    mu = np.asarray(bn_mean, np.float64)
    var = np.asarray(bn_var, np.float64)

    inv = g / np.sqrt(var + EPS)
    # SCALE (softmax(q) * HD**-0.5) is folded into the output projection
    w_out_f = (w_out.astype(np.float64) * inv[:, None] * SCALE).astype(np.float32)
    b_out_f = ((b_out.astype(np.float64) - mu) * inv + be).astype(np.float32)

    consts = {}
    consts["wqkvT"] = np.ascontiguousarray(w_qkv.T.reshape(NCT, 128, 3 * MD)).astype(np.float16)
    consts["woutT"] = np.ascontiguousarray(w_out_f.T.reshape(NDT, 128, C)).astype(np.float16)
    consts["bq"] = np.ascontiguousarray(b_qkv[0:MD].reshape(NDT, 128).T)
    consts["bout"] = np.ascontiguousarray(b_out_f.reshape(2, 128).T)

    bones = np.zeros((128, 128), np.float16)
    bones[0:64, 0:64] = 1.0
    bones[64:128, 64:128] = 1.0
    consts["bones"] = bones

    mk = mem_kv[0].astype(np.float64)
    mv = mem_kv[1].astype(np.float64)
    emk = np.exp(mk)
    ctx_mem = np.einsum("him,hjm->hij", emk, mv)
    z_mem = emk.sum(-1)
    cmem = np.zeros((128, NDT, 258), np.float16)
    for p in range(NDT):
        base = (p % 2) * 128
        for t in range(2):
            h = 2 * p + t
            r0 = 64 * t
            cmem[r0:r0 + 64, p, base + 64 * t: base + 64 * t + 64] = ctx_mem[h]
            cmem[r0:r0 + 64, p, 256] = z_mem[h]
    consts["cmem"] = cmem
    consts["ident"] = np.eye(128, dtype=np.float16)

    has_bk = bool(np.any(b_qkv[MD:2 * MD] != 0))
    has_bv = bool(np.any(b_qkv[2 * MD:] != 0))
    if has_bk or has_bv:
        # K=2 rank-2 form (fp32r wants even dims): ones row + zero row
        onesrow = np.zeros((2, 128), np.float16)
        onesrow[0] = 1.0
        consts["onesrow"] = onesrow
        bkv = np.zeros((2, 2 * MD), np.float16)
        bkv[0] = b_qkv[MD:].astype(np.float16)
        consts["bkv"] = bkv
    return consts, has_bk, has_bv


def kernel(x, w_qkv, b_qkv, mem_kv, w_out, b_out, bn_gamma, bn_beta, bn_mean, bn_var):
    x = np.asarray(x, np.float32)
    consts, has_bk, has_bv = _prep_consts(
        w_qkv, b_qkv, mem_kv, w_out, b_out, bn_gamma, bn_beta, bn_mean, bn_var
    )

    key = (has_bk, has_bv)
    if key not in _MODULE_CACHE:
        _MODULE_CACHE[key] = _build_module(has_bk, has_bv)
    nc = _MODULE_CACHE[key]

    x_t = x.reshape(B, NCT, 128, S).astype(np.float16)
    in_maps = []
    for c in range(N_CORES):
        m = dict(consts)
        m["x"] = np.ascontiguousarray(x_t[c * BPC:(c + 1) * BPC])
        in_maps.append(m)

    trace = bool(int(os.environ.get("BASS_KERNEL_TRACE", "0")))
    res = bass_utils.run_bass_kernel_spmd(
        nc, in_maps, core_ids=list(range(N_CORES)), trace=trace
    )
    if trace:
        kernel.last_exec_time_ns = res.exec_time_ns
        kernel.last_mean_exec_time_ns = res.mean_exec_time_ns

    y = np.stack([res.results[c]["y"] for c in range(N_CORES)])
    y = y.reshape(B, C, H, W).astype(np.float32)
    return y


# revision 17
# speedup vs baseline: 1.1935x; 1.1935x over previous
"""Trainium2 Bass kernel for LinearSelfAttention3D (16x256x64x64, 8 heads, mem_kv).

Data-parallel over batch: 2 batches per core, 8 cores, identical SPMD program.
Per batch (x viewed [256, 4096] channel-major):
  Pass A (32 s-tiles of 128, ctx matmuls software-pipelined one tile behind):
    kT,vT = x^T @ w_{k,v}^T on PE (lhsT = x c-tiles -> [s,d] layout, zero transposes)
    expk = exp(kT) (ACT, fp32r); vT staged with ones-cols (DVE)
    context accumulated in PSUM: 4 head-pair tiles [128, 258]
      (2 heads per tile; col 256 accumulates Z = sum_s expk)
    mem_kv folded on host; added via one identity-matmul [ctxU_mem | Zmem]
    evac: ctx_diag[p] = blockdiag(ctx/Z) fp32r SBUF (batched approx reciprocal)
  Pass B (8 s-chunks of 512, second half pipelined one chunk behind):
    q = w_q @ x (PE, [d,s] layout); expU = exp(q) (ACT)
    Zq[h,s] via indicator matmul; 1/Zq via reciprocal_approx_fast (DVE)
    Bcast = SCALE/Zq broadcast over rows via selector matmul (PE)
    expq = expU * Bcast (DVE); attn_out = ctx_diag^T @ expq (PE)
    y = w_out'^T.T @ attn_out + b' (PE; BatchNorm folded into w_out'/b' on host)
All matmuls fp16 (1 cycle/row, FWL weight loads, ~1.5e-3 rel err).
"""
import os
import sys

sys.path.insert(0, "/opt/trn_rl_repo")
import numpy as np

import concourse.bass as bass  # noqa: E402
import concourse.bacc as bacc  # noqa: E402
import concourse.mybir as mybir  # noqa: E402
import concourse.tile as tile  # noqa: E402
from concourse import bass_utils  # noqa: E402

B, C, H, W = 16, 256, 64, 64
S = H * W  # 4096
MD, NH, HD, NM = 512, 8, 64, 4
SCALE = HD ** -0.5
EPS = 1e-5
N_CORES = 8
BPC = B // N_CORES
NCT = C // 128
NST = S // 128
NSC = S // 512
NDT = MD // 128
F32 = mybir.dt.float32
F16 = mybir.dt.float16
AF = mybir.ActivationFunctionType

_MODULE_CACHE = {}


def _build_module(has_bk, has_bv):
    nc = bacc.Bacc(
        "TRN2",
        target_bir_lowering=False,
        debug=False,
        enable_asserts=False,
        num_devices=N_CORES,
    )
    x_d = nc.dram_tensor("x", (BPC, NCT, 128, S), F16, kind="ExternalInput").ap()
    wqkvT_d = nc.dram_tensor("wqkvT", (NCT, 128, 3 * MD), F16, kind="ExternalInput").ap()
    woutT_d = nc.dram_tensor("woutT", (NDT, 128, C), F16, kind="ExternalInput").ap()
    bq_d = nc.dram_tensor("bq", (128, NDT), F32, kind="ExternalInput").ap()
    bout_d = nc.dram_tensor("bout", (128, 2), F32, kind="ExternalInput").ap()
    bones_d = nc.dram_tensor("bones", (128, 128), F16, kind="ExternalInput").ap()
    cmem_d = nc.dram_tensor("cmem", (128, NDT, 258), F16, kind="ExternalInput").ap()
    ident_d = nc.dram_tensor("ident", (128, 128), F16, kind="ExternalInput").ap()
    y_d = nc.dram_tensor("y", (BPC, 2, 128, S), F16, kind="ExternalOutput").ap()
    if has_bk or has_bv:
        onesrow_d = nc.dram_tensor("onesrow", (2, 128), F16, kind="ExternalInput").ap()
        bkv_d = nc.dram_tensor("bkv", (2, 2 * MD), F16, kind="ExternalInput").ap()

    with tile.TileContext(nc) as tc, nc.allow_low_precision(reason="fp16 matmul operands"):
        import contextlib

        cstack = contextlib.ExitStack()
        const = cstack.enter_context(tc.tile_pool(name="const", bufs=1))
        xrp = cstack.enter_context(tc.tile_pool(name="xrp", bufs=2))
        work = cstack.enter_context(tc.tile_pool(name="work", bufs=3))
        ctxdp = cstack.enter_context(tc.tile_pool(name="ctxdp", bufs=8))
        pool9 = cstack.enter_context(tc.tile_pool(name="pool9", bufs=9))
        pool4 = cstack.enter_context(tc.tile_pool(name="pool4", bufs=4))
        pool2 = cstack.enter_context(tc.tile_pool(name="pool2", bufs=2))

        def load_r(shape, src_ap, tag, n_splits=None):
            t = const.tile(list(shape), F16, tag=tag, name=tag)
            if n_splits is None:
                nc.sync.dma_start(t[:], src_ap)
            else:
                for i in range(n_splits):
                    nc.sync.dma_start(t[:, i], src_ap[i])
            return t

        wq_r = load_r((128, NCT, 3 * MD), wqkvT_d, "wq", n_splits=NCT)
        wo_r = load_r((128, NDT, C), woutT_d, "wo", n_splits=NDT)
        bones_r = load_r((128, 128), bones_d, "bones")
        cmem_r = load_r((128, NDT, 258), cmem_d, "cmem")
        ident_r = load_r((128, 128), ident_d, "ident")
        if has_bk or has_bv:
            onesrow_r = load_r((2, 128), onesrow_d, "onesrow")
            bkv_r = load_r((2, 2 * MD), bkv_d, "bkv")

        bq_t = const.tile([128, NDT], F32, tag="bq")
        nc.sync.dma_start(bq_t[:], bq_d)
        bout_t = const.tile([128, 2], F32, tag="bout")
        nc.sync.dma_start(bout_t[:], bout_d)

        zero_r = const.tile([128, 128], F16, tag="zeror")
        nc.vector.memset(zero_r[:], 0.0)
        ones_r = const.tile([128, 4], F16, tag="onesr")
        nc.vector.memset(ones_r[:], 1.0)

        wzero = const.tile([128, 512], F16, tag="wzero")
        nc.vector.memset(wzero[:], 0.0)
        # HAM warmup: dense matmuls with no DMA dependency (overlap input DMAs)
        with tc.tile_pool(name="warm", bufs=1, space="PSUM") as pw:
            wps = pw.tile([128, 512], F32, tag="warm")
            for i in range(24):
                nc.tensor.matmul(wps[:], zero_r[:], wzero[:],
                                 start=True, stop=True)

        for b in range(BPC):
            x_r = xrp.tile([128, NCT, S], F16, tag="xr")
            for ct in range(NCT):
                nc.sync.dma_start(x_r[:, ct, :], x_d[b, ct])

            # ---- pass A ----
            ctxd = []
            with tc.tile_pool(name=f"pp{b}", bufs=1, space="PSUM") as pp:
                ctxps = [
                    pp.tile([128, 258], F32, tag=f"s{p}", name=f"ctx{p}_{b}")
                    for p in range(NDT)
                ]

                def ctx_mms(ek, vt, st):
                    for p in range(NDT):
                        nc.tensor.matmul(
                            ctxps[p][:],
                            ek[:, p * 128:(p + 1) * 128],
                            vt[:, p // 2, :],
                            start=(st == 0),
                            stop=False,
                        )

                prev = None
                for st in range(NST):
                    kps = pp.tile([128, 512], F32, tag=f"s{4 + (2 * st) % 4}",
                                  name=f"k_{b}_{st}")
                    vps = pp.tile([128, 512], F32, tag=f"s{4 + (2 * st + 1) % 4}",
                                  name=f"v_{b}_{st}")
                    # interleaved so consecutive matmuls share the stationary x tile
                    for ct in range(NCT):
                        last = ct == NCT - 1
                        nc.tensor.matmul(
                            kps[:],
                            x_r[:, ct, st * 128:(st + 1) * 128],
                            wq_r[:, ct, MD:2 * MD],
                            start=(ct == 0),
                            stop=(last and not has_bk),
                        )
                        nc.tensor.matmul(
                            vps[:],
                            x_r[:, ct, st * 128:(st + 1) * 128],
                            wq_r[:, ct, 2 * MD:3 * MD],
                            start=(ct == 0),
                            stop=(last and not has_bv),
                        )
                    if has_bk:
                        nc.tensor.matmul(kps[:], onesrow_r[:], bkv_r[:, 0:MD],
                                         start=False, stop=True)
                    if has_bv:
                        nc.tensor.matmul(vps[:], onesrow_r[:], bkv_r[:, MD:2 * MD],
                                         start=False, stop=True)
                    if prev is not None:
                        ctx_mms(*prev)
                    ek = work.tile([128, 512], F16, tag="ek")
                    nc.scalar.activation(ek[:], kps[:], AF.Exp)
                    vt = work.tile([128, 2, 258], F16, tag="vt")
                    nc.vector.tensor_copy(
                        vt[:, :, 0:256], vps[:].rearrange("p (g c) -> p g c", g=2)
                    )
                    nc.vector.tensor_copy(
                        vt[:, :, 256:258], ones_r[:].rearrange("p (g c) -> p g c", g=2)
                    )
                    prev = (ek, vt, st)
                ctx_mms(*prev)
                for p in range(NDT):
                    nc.tensor.matmul(ctxps[p][:], ident_r[:], cmem_r[:, p, :],
                                     start=False, stop=True)
                # ctx evac: batched 1/Z then per-head blockdiag scale
                zcat = pool4.tile([128, 4], F32, tag="zcat")
                for p in range(NDT):
                    nc.vector.tensor_copy(zcat[:, p:p + 1], ctxps[p][:, 256:257])
                rz = pool4.tile([128, 4], F32, tag="rz")
                nc.vector.reciprocal_approx_fast(rz[:], zcat[:])
                for p in range(NDT):
                    cd = ctxdp.tile([128, 128], F16, tag="ctxd", name=f"cd{p}_{b}")
                    base = (p % 2) * 128
                    nc.vector.tensor_copy(cd[0:64, 64:128], zero_r[0:64, 0:64])
                    nc.vector.tensor_copy(cd[64:128, 0:64], zero_r[0:64, 0:64])
                    nc.vector.tensor_scalar_mul(
                        cd[0:64, 0:64], ctxps[p][0:64, base:base + 64], rz[0:64, p:p + 1]
                    )
                    nc.vector.tensor_scalar_mul(
                        cd[64:128, 64:128],
                        ctxps[p][64:128, base + 64:base + 128],
                        rz[64:128, p:p + 1],
                    )
                    ctxd.append(cd)
                # ---- pass B ---- (3-stage pipeline: [q+stats] -> [attn out] -> [y])

                def stage_ao(sc, eus, rbs):
                    outs = []
                    for d in range(NDT):
                        ao = pp.tile([128, 512], F32, tag=f"s{d % 2}",
                                     name=f"ao{b}_{sc}_{d}")
                        nc.tensor.matmul(ao[:], ctxd[d][:], eus[d][:],
                                         start=True, stop=True)
                        osb = pool9.tile([128, 512], F16, tag="aosb")
                        nc.vector.tensor_mul(osb[:], rbs[d][:], ao[:])
                        outs.append(osb)
                    return outs

                def stage_y(sc, outs):
                    s0 = sc * 512
                    for ot in range(2):
                        yps = pp.tile([128, 512], F32, tag=f"s{7 - 4 * ot}",
                                      name=f"y{b}_{sc}_{ot}")
                        for j in range(NDT):
                            nc.tensor.matmul(
                                yps[:],
                                wo_r[:, j, ot * 128:(ot + 1) * 128],
                                outs[j][:],
                                start=(j == 0),
                                stop=(j == NDT - 1),
                            )
                        ysb = pool4.tile([128, 512], F16, tag="ysb")
                        nc.scalar.activation(ysb[:], yps[:], AF.Identity,
                                             bias=bout_t[:, ot:ot + 1])
                        nc.sync.dma_start(y_d[b, ot, :, s0:s0 + 512], ysb[:])

                st1 = None  # (sc, eus, rbs) awaiting attn-out
                st2 = None  # (sc, outs) awaiting y projection
                for sc in range(NSC):
                    s0 = sc * 512
                    eus = []
                    rbs = []
                    for d in range(NDT):
                        qps = pp.tile([128, 512], F32, tag=f"s{5 + d % 2}",
                                      name=f"q{b}_{sc}_{d}")
                        for ct in range(NCT):
                            nc.tensor.matmul(
                                qps[:],
                                wq_r[:, ct, d * 128:(d + 1) * 128],
                                x_r[:, ct, s0:s0 + 512],
                                start=(ct == 0),
                                stop=(ct == NCT - 1),
                            )
                        eu = pool9.tile([128, 512], F16, tag="eu")
                        nc.scalar.activation(eu[:], qps[:], AF.Exp,
                                             bias=bq_t[:, d:d + 1])
                        # Zq in row-broadcast layout via block-diag ones matmul
                        zqb = pp.tile([128, 512], F32, tag=f"s{2 + 2 * (d % 2)}",
                                      name=f"zqb{b}_{sc}_{d}")
                        nc.tensor.matmul(zqb[:], bones_r[:], eu[:],
                                         start=True, stop=True)
                        rb = pool9.tile([128, 512], F32, tag="rb")
                        nc.vector.reciprocal_approx_fast(rb[:], zqb[:])
                        eus.append(eu)
                        rbs.append(rb)
                    if st2 is not None:
                        stage_y(*st2)
                    if st1 is not None:
                        st2 = (st1[0], stage_ao(*st1))
                    st1 = (sc, eus, rbs)
                if st2 is not None:
                    stage_y(*st2)
                st2 = (st1[0], stage_ao(*st1))
                stage_y(*st2)
        cstack.close()

    nc.compile()
    return nc


def _prep_consts(w_qkv, b_qkv, mem_kv, w_out, b_out, bn_gamma, bn_beta, bn_mean, bn_var):
    w_qkv = np.asarray(w_qkv, np.float32)
    b_qkv = np.asarray(b_qkv, np.float32)
    mem_kv = np.asarray(mem_kv, np.float32)
    w_out = np.asarray(w_out, np.float32)
    b_out = np.asarray(b_out, np.float32)
    g = np.asarray(bn_gamma, np.float64)
    be = np.asarray(bn_beta, np.float64)
    mu = np.asarray(bn_mean, np.float64)
    var = np.asarray(bn_var, np.float64)

    inv = g / np.sqrt(var + EPS)
    # SCALE (softmax(q) * HD**-0.5) is folded into the output projection
    w_out_f = (w_out.astype(np.float64) * inv[:, None] * SCALE).astype(np.float32)
    b_out_f = ((b_out.astype(np.float64) - mu) * inv + be).astype(np.float32)

    consts = {}
    consts["wqkvT"] = np.ascontiguousarray(w_qkv.T.reshape(NCT, 128, 3 * MD)).astype(np.float16)
    consts["woutT"] = np.ascontiguousarray(w_out_f.T.reshape(NDT, 128, C)).astype(np.float16)
    consts["bq"] = np.ascontiguousarray(b_qkv[0:MD].reshape(NDT, 128).T)
    consts["bout"] = np.ascontiguousarray(b_out_f.reshape(2, 128).T)

    bones = np.zeros((128, 128), np.float16)
    bones[0:64, 0:64] = 1.0
    bones[64:128, 64:128] = 1.0
    consts["bones"] = bones

    mk = mem_kv[0].astype(np.float64)
    mv = mem_kv[1].astype(np.float64)
    emk = np.exp(mk)
    ctx_mem = np.einsum("him,hjm->hij", emk, mv)
    z_mem = emk.sum(-1)
    cmem = np.zeros((128, NDT, 258), np.float16)
    for p in range(NDT):
        base = (p % 2) * 128
        for t in range(2):
            h = 2 * p + t
            r0 = 64 * t
            cmem[r0:r0 + 64, p, base + 64 * t: base + 64 * t + 64] = ctx_mem[h]
            cmem[r0:r0 + 64, p, 256] = z_mem[h]
    consts["cmem"] = cmem
    consts["ident"] = np.eye(128, dtype=np.float16)

    has_bk = bool(np.any(b_qkv[MD:2 * MD] != 0))
    has_bv = bool(np.any(b_qkv[2 * MD:] != 0))
    if has_bk or has_bv:
        # K=2 rank-2 form (fp32r wants even dims): ones row + zero row
        onesrow = np.zeros((2, 128), np.float16)
        onesrow[0] = 1.0
        consts["onesrow"] = onesrow
        bkv = np.zeros((2, 2 * MD), np.float16)
        bkv[0] = b_qkv[MD:].astype(np.float16)
        consts["bkv"] = bkv
    return consts, has_bk, has_bv


def kernel(x, w_qkv, b_qkv, mem_kv, w_out, b_out, bn_gamma, bn_beta, bn_mean, bn_var):
    x = np.asarray(x, np.float32)
    consts, has_bk, has_bv = _prep_consts(
        w_qkv, b_qkv, mem_kv, w_out, b_out, bn_gamma, bn_beta, bn_mean, bn_var
    )

    key = (has_bk, has_bv)
    if key not in _MODULE_CACHE:
        _MODULE_CACHE[key] = _build_module(has_bk, has_bv)
    nc = _MODULE_CACHE[key]

    x_t = x.reshape(B, NCT, 128, S).astype(np.float16)
    in_maps = []
    for c in range(N_CORES):
        m = dict(consts)
        m["x"] = np.ascontiguousarray(x_t[c * BPC:(c + 1) * BPC])
        in_maps.append(m)

    trace = bool(int(os.environ.get("BASS_KERNEL_TRACE", "0")))
    res = bass_utils.run_bass_kernel_spmd(
        nc, in_maps, core_ids=list(range(N_CORES)), trace=trace
    )
    if trace:
        kernel.last_exec_time_ns = res.exec_time_ns
        kernel.last_mean_exec_time_ns = res.mean_exec_time_ns

    y = np.stack([res.results[c]["y"] for c in range(N_CORES)])
    y = y.reshape(B, C, H, W).astype(np.float32)
    return y
